# revision 7
# baseline (speedup 1.0000x reference)
"""MBD degradation-imputation sampling step on 8 Trainium2 NeuronCores.

Strategy (data-parallel over the N=2048 candidate samples, 256/core):
  pass A : per-sample consistency scores, one HBM pass over eps.
           Observed positions are made sample-independent by a
           host-prepared additive tensor c0 that saturates the clip
           (softmax is shift-invariant so the constant contribution
           cancels):
               u  = eps + c0            (DVE tensor_tensor, f32)
               vh = clip(u, +-1/sigma)  (DVE tensor_scalar -> fp16 CACHE)
               d  = vh - q'             (Pool tensor_tensor, fp16)
               S += sum(d^2)            (ACT Square + accum_out)
           The fp16 clipped values stay resident in SBUF (128 KiB/pn)
           so pass B never re-reads eps.
  AllGather the 2048 scores (8 KB), softmax stats on-device, each core
  weights its local samples.
  pass B : weighted partition-reduction straight out of the fp16 SBUF
           cache on the TensorEngine (fp16 matmuls, M=1, PSUM-
           accumulated), AllReduce the (T,F) partials (128 KB), final
           mask-select against observed_data.

`stage` truncates the program for hardware bisection:
  1 = pass A only, 2 = +AllGather/softmax, 3 = +pass B (no AllReduce),
  4 = full kernel.
"""

from contextlib import ExitStack

import numpy as np

import concourse.bass as bass
import concourse.tile as tile
from concourse import bacc, mybir
from concourse.bass_utils import run_bass_kernel_spmd

N_CORES = 8
N, T, F = 2048, 512, 64
P = 128
TF = T * F                      # 32768
NLOC = N // N_CORES             # 256
NBLK = NLOC // P                # 2
CHUNK = 1024
NCHUNK = TF // CHUNK            # 32
SUB = 512                       # matmul N (one PSUM bank)
TEMP = 0.1
T_STEPS = 1000

F32 = mybir.dt.float32
F16 = mybir.dt.float16
AX = mybir.AxisListType
ALU = mybir.AluOpType
ACTF = mybir.ActivationFunctionType


def _schedule_scalars(i: int):
    s = 0.008
    x = np.linspace(0, T_STEPS, T_STEPS + 1, dtype=np.float64)
    ac = np.cos((x / T_STEPS + s) / (1 + s) * np.pi * 0.5) ** 2
    ac = ac / ac[0]
    betas = np.clip(1.0 - ac[1:] / ac[:-1], 0.0, 0.999)
    alphas = 1.0 - betas
    acp = np.cumprod(alphas)
    abar_i = np.float32(acp[i])
    sigma_i = np.float32(np.sqrt(1.0 - acp[i]))
    alpha_i = np.float32(alphas[i])
    abar_im1 = np.float32(acp[i - 1])
    sa = np.float32(np.sqrt(abar_i))
    # the reference's Yi terms cancel exactly; out_missing = c1 * weighted
    c1 = np.float32(sa / np.float32(np.sqrt(alpha_i)) / np.float32(np.sqrt(abar_im1)))
    return sigma_i, c1


def _build(sigma_i: float, c1: float, stage: int = 4):
    inv_sig = float(np.float32(1.0 / np.float32(sigma_i)))
    sigma_i = float(np.float32(sigma_i))
    c1 = float(np.float32(c1))
    # scores = cA * sum((v - q')^2)  (+ sample-independent shift vs ref)
    cA = float(np.float32(-(np.float32(sigma_i) ** 2) / np.float32(TF)))

    nc = bacc.Bacc(
        "TRN2", target_bir_lowering=False, debug=False, num_devices=N_CORES
    )
    eps_d = nc.dram_tensor("eps", [NLOC, TF], F32, kind="ExternalInput")
    c0_d = nc.dram_tensor("c0", [TF], F32, kind="ExternalInput")
    qp_d = nc.dram_tensor("qp", [TF], F16, kind="ExternalInput")
    obs_d = nc.dram_tensor("obs", [TF], F32, kind="ExternalInput")
    maskf_d = nc.dram_tensor("maskf", [TF], F32, kind="ExternalInput")
    out_d = nc.dram_tensor("out", [TF], F32, kind="ExternalOutput")

    sc_loc_d = nc.dram_tensor("sc_loc", [NLOC], F32)
    sc_all_d = nc.dram_tensor("sc_all", [N], F32, addr_space="Shared")
    ws_loc_d = nc.dram_tensor("ws_loc", [TF], F32)
    ws_all_d = nc.dram_tensor("ws_all", [TF], F32, addr_space="Shared")

    rg = [list(range(N_CORES))]

    with tile.TileContext(nc) as tc, ExitStack() as ctx:
        eps_ap = eps_d.ap()

        rows = ctx.enter_context(tc.tile_pool(name="rows", bufs=2))
        rowsq = ctx.enter_context(tc.tile_pool(name="rowsq", bufs=2))
        epsp = ctx.enter_context(tc.tile_pool(name="epsp", bufs=3))
        work = ctx.enter_context(tc.tile_pool(name="work", bufs=2))
        workh = ctx.enter_context(tc.tile_pool(name="workh", bufs=3))
        cache = ctx.enter_context(tc.tile_pool(name="cache", bufs=1))
        stat = ctx.enter_context(tc.tile_pool(name="stat", bufs=1))
        smal = ctx.enter_context(tc.tile_pool(name="smal", bufs=1))
        psum = ctx.enter_context(tc.tile_pool(name="psum", bufs=1, space="PSUM"))

        # fp16 clipped-values cache: 64 tiles of [128, 1024] packed into
        # one persistent tile (128 KiB per partition)
        vcache = cache.tile([P, NBLK * NCHUNK * CHUNK], F16, tag="vc",
                            name="vcache")

        # ---------------- pass A: local scores ----------------
        sa_cols = [
            stat.tile([P, NCHUNK], F32, tag=f"sa{b}", name=f"sa_cols{b}")
            for b in range(NBLK)
        ]
        for k in range(NCHUNK):
            sl = slice(k * CHUNK, (k + 1) * CHUNK)
            c0_t = rows.tile([P, CHUNK], F32, tag="c0", name="c0_t")
            nc.gpsimd.dma_start(out=c0_t[:], in_=c0_d.ap()[sl].partition_broadcast(P))
            q_t = rowsq.tile([P, CHUNK], F16, tag="q", name="q_t")
            nc.scalar.dma_start(out=q_t[:], in_=qp_d.ap()[sl].partition_broadcast(P))
            for b in range(NBLK):
                off = (k * NBLK + b) * CHUNK
                e_t = epsp.tile([P, CHUNK], F32, tag="eps", name="e_t")
                nc.sync.dma_start(out=e_t[:], in_=eps_ap[b * P:(b + 1) * P, sl])
                u_t = work.tile([P, CHUNK], F32, tag="u", name="u_t")
                nc.vector.tensor_tensor(
                    out=u_t[:], in0=e_t[:], in1=c0_t[:], op=ALU.add
                )
                vsl = vcache[:, off:off + CHUNK]
                nc.vector.tensor_scalar(
                    out=vsl, in0=u_t[:], scalar1=inv_sig, scalar2=-inv_sig,
                    op0=ALU.min, op1=ALU.max,
                )
                d_t = workh.tile([P, CHUNK], F16, tag="d", name="d_t")
                nc.gpsimd.tensor_tensor(
                    out=d_t[:], in0=vsl, in1=q_t[:], op=ALU.subtract
                )
                nc.scalar.activation(
                    out=d_t[:], in_=d_t[:], func=ACTF.Square,
                    accum_out=sa_cols[b][:, k:k + 1],
                )

        s_loc = stat.tile([P, NBLK], F32, tag="sloc", name="s_loc")
        for b in range(NBLK):
            sa_tot = smal.tile([P, 1], F32, tag="sat", name="sa_tot")
            nc.vector.tensor_reduce(sa_tot[:], sa_cols[b][:], axis=AX.X, op=ALU.add)
            nc.vector.tensor_scalar_mul(s_loc[:, b:b + 1], sa_tot[:], cA)
        nc.sync.dma_start(
            out=sc_loc_d.ap().rearrange("(b p) -> p b", p=P), in_=s_loc[:]
        )
        if stage <= 1:
            nc.sync.dma_start(
                out=out_d.ap()[0:NLOC].rearrange("(b p) -> p b", p=P),
                in_=s_loc[:],
            )

        # ---------------- gather scores, softmax stats ----------------
        wt16 = None
        if stage >= 2:
            nc.gpsimd.collective_compute(
                "AllGather", ALU.bypass,
                ins=[sc_loc_d.ap()], outs=[sc_all_d.ap()], replica_groups=rg,
            )
            s_all = smal.tile([1, N], F32, tag="sall", name="s_all")
            nc.sync.dma_start(
                out=s_all[:], in_=sc_all_d.ap().rearrange("(a n) -> a n", a=1)
            )
            negmean = smal.tile([1, 1], F32, tag="negmean", name="negmean")
            nc.vector.tensor_reduce(negmean[:], s_all[:], axis=AX.X, op=ALU.add)
            nc.vector.tensor_scalar_mul(negmean[:], negmean[:], -1.0 / N)
            mean = smal.tile([1, 1], F32, tag="mean", name="mean")
            nc.vector.tensor_scalar_mul(mean[:], negmean[:], -1.0)
            js = smal.tile([1, N], F32, tag="js", name="js")
            ssq = smal.tile([1, 1], F32, tag="ssq", name="ssq")
            nc.scalar.activation(
                out=js[:], in_=s_all[:], func=ACTF.Square, bias=negmean[:]
            )
            nc.vector.tensor_reduce(ssq[:], js[:], axis=AX.X, op=ALU.add)
            var = smal.tile([1, 1], F32, tag="var", name="var")
            nc.vector.tensor_scalar_mul(var[:], ssq[:], 1.0 / (N - 1))
            std = smal.tile([1, 1], F32, tag="std", name="std")
            nc.scalar.activation(out=std[:], in_=var[:], func=ACTF.Sqrt)
            nc.vector.tensor_scalar_max(std[:], std[:], 1e-4)
            inv10 = smal.tile([1, 1], F32, tag="inv10", name="inv10")
            nc.vector.reciprocal(inv10[:], std[:])
            nc.vector.tensor_scalar_mul(inv10[:], inv10[:], 1.0 / TEMP)
            mx = smal.tile([1, 1], F32, tag="mx", name="mx")
            nc.vector.tensor_reduce(mx[:], s_all[:], axis=AX.X, op=ALU.max)
            bg = smal.tile([1, 1], F32, tag="bg", name="bg")
            nc.vector.scalar_tensor_tensor(
                out=bg[:], in0=mean[:], scalar=inv10[:], in1=mx[:],
                op0=ALU.mult, op1=ALU.add,
            )
            nc.vector.tensor_scalar_mul(bg[:], bg[:], -1.0)
            je = smal.tile([1, N], F32, tag="je", name="je")
            zsum = smal.tile([1, 1], F32, tag="zsum", name="zsum")
            nc.scalar.activation(
                out=je[:], in_=s_all[:], func=ACTF.Exp, scale=inv10[:], bias=bg[:]
            )
            nc.vector.tensor_reduce(zsum[:], je[:], axis=AX.X, op=ALU.add)
            rz = smal.tile([1, 1], F32, tag="rz", name="rz")
            nc.vector.reciprocal(rz[:], zsum[:])

            # broadcast the 3 softmax scalars to 128 partitions via DRAM
            pack = smal.tile([1, 3], F32, tag="pack", name="pack")
            nc.vector.tensor_copy(pack[:, 0:1], inv10[:])
            nc.vector.tensor_copy(pack[:, 1:2], bg[:])
            nc.vector.tensor_copy(pack[:, 2:3], rz[:])
            pk_d = nc.dram_tensor("pk", [3], F32)
            nc.sync.dma_start(
                out=pk_d.ap().rearrange("(a n) -> a n", a=1), in_=pack[:]
            )
            scal = smal.tile([P, 3], F32, tag="scal", name="scal")
            nc.sync.dma_start(out=scal[:], in_=pk_d.ap()[0:3].partition_broadcast(P))

            e_loc = smal.tile([P, NBLK], F32, tag="eloc", name="e_loc")
            nc.scalar.activation(
                out=e_loc[:], in_=s_loc[:], func=ACTF.Exp,
                scale=scal[:, 0:1], bias=scal[:, 1:2],
            )
            wt = stat.tile([P, NBLK], F32, tag="wt", name="wt")
            nc.vector.tensor_scalar(
                out=wt[:], in0=e_loc[:], scalar1=scal[:, 2:3], scalar2=sigma_i,
                op0=ALU.mult, op1=ALU.mult,
            )
            wt16 = stat.tile([P, NBLK], F16, tag="wt16", name="wt16")
            nc.scalar.copy(wt16[:], wt[:])
            if stage <= 2:
                nc.sync.dma_start(
                    out=out_d.ap()[0:NLOC].rearrange("(b p) -> p b", p=P),
                    in_=wt[:],
                )

        # ---------------- pass B: weighted sum on PE from SBUF cache ----
        if stage >= 3:
            for s in range(TF // SUB):
                k, half = s // 2, s % 2
                wrow = psum.tile([1, SUB], F32, tag="wrow", bufs=4, name="wrow")
                for b in range(NBLK):
                    off = (k * NBLK + b) * CHUNK + half * SUB
                    nc.tensor.matmul(
                        wrow[:], lhsT=wt16[:, b:b + 1],
                        rhs=vcache[:, off:off + SUB],
                        start=(b == 0), stop=(b == NBLK - 1),
                    )
                wsb = work.tile([1, SUB], F32, tag="wsb", bufs=4, name="wsb")
                nc.vector.tensor_copy(wsb[:], wrow[:])
                nc.sync.dma_start(
                    out=ws_loc_d.ap()[s * SUB:(s + 1) * SUB]
                    .rearrange("(a n) -> a n", a=1),
                    in_=wsb[:],
                )
            if stage <= 3:
                o3 = stat.tile([P, TF // P], F32, tag="o3", name="o3")
                nc.sync.dma_start(
                    out=o3[:], in_=ws_loc_d.ap().rearrange("(p c) -> p c", p=P)
                )
                nc.sync.dma_start(
                    out=out_d.ap().rearrange("(p c) -> p c", p=P), in_=o3[:]
                )

        # ---------------- AllReduce + final combine ----------------
        if stage >= 4:
            nc.gpsimd.collective_compute(
                "AllReduce", ALU.add,
                ins=[ws_loc_d.ap()], outs=[ws_all_d.ap()], replica_groups=rg,
            )
            rowmaj = lambda d: d.ap().rearrange("(p c) -> p c", p=P)
            w_t = stat.tile([P, TF // P], F32, tag="wfin", name="w_t")
            nc.sync.dma_start(out=w_t[:], in_=rowmaj(ws_all_d))
            obs_t = stat.tile([P, TF // P], F32, tag="obsf", name="obs_t")
            nc.sync.dma_start(out=obs_t[:], in_=rowmaj(obs_d))
            m_t = stat.tile([P, TF // P], F32, tag="mf", name="m_t")
            nc.sync.dma_start(out=m_t[:], in_=rowmaj(maskf_d))
            t1 = stat.tile([P, TF // P], F32, tag="t1", name="t1")
            nc.vector.tensor_scalar_mul(t1[:], w_t[:], c1)
            t2 = stat.tile([P, TF // P], F32, tag="t2", name="t2")
            nc.vector.tensor_tensor(t2[:], obs_t[:], t1[:], ALU.subtract)
            t3 = stat.tile([P, TF // P], F32, tag="t3", name="t3")
            nc.vector.tensor_tensor(t3[:], t2[:], m_t[:], ALU.mult)
            o_t = stat.tile([P, TF // P], F32, tag="ot", name="o_t")
            nc.vector.tensor_tensor(o_t[:], t1[:], t3[:], ALU.add)
            nc.sync.dma_start(out=rowmaj(out_d), in_=o_t[:])

    nc.compile()
    return nc


_CACHE: dict = {}
TRACE = False
STAGE = 4
LAST_RESULTS = None


def kernel(Xbar_i, observed_data, time_points, mask, eps, deg_a, deg_b, i):
    global LAST_RESULTS
    i = int(i)
    sigma_i, c1 = _schedule_scalars(i)
    key = ("v2", i, STAGE)
    if key not in _CACHE:
        _CACHE[key] = _build(float(sigma_i), float(c1), stage=STAGE)
    nc = _CACHE[key]

    inv_sig = np.float32(1.0) / sigma_i
    Xb = np.asarray(Xbar_i, np.float32)
    obs = np.asarray(observed_data, np.float32)
    msk = np.asarray(mask, bool)
    tp = np.asarray(time_points, np.float32)
    da = np.asarray(deg_a, np.float32)
    db = np.asarray(deg_b, np.float32)
    epsf = np.asarray(eps, np.float32)

    pred = da[None, :] + db[None, :] * tp[:, None]
    c0 = (Xb * inv_sig).astype(np.float32)
    c0 = np.where(msk, np.float32(1e6), c0).reshape(-1)
    qp = (pred * inv_sig).astype(np.float32)
    qp = np.where(msk, inv_sig, qp).reshape(-1).astype(np.float16)
    obsf = obs.reshape(-1)
    maskf = msk.astype(np.float32).reshape(-1)

    in_maps = []
    for c in range(N_CORES):
        shard = np.ascontiguousarray(
            epsf[c * NLOC:(c + 1) * NLOC].reshape(NLOC, TF)
        )
        in_maps.append(
            {"eps": shard, "c0": c0, "qp": qp, "obs": obsf, "maskf": maskf}
        )
    kr = run_bass_kernel_spmd(nc, in_maps, list(range(N_CORES)), trace=TRACE)
    LAST_RESULTS = kr
    return kr.results[0]["out"].reshape(T, F).astype(np.float32)
